# revision 1
# baseline (speedup 1.0000x reference)
"""AuxLossFreeMoE TRN2 kernel: 16-expert top-2 sigmoid-gated MoE + shared expert.

Strategy (8 NeuronCores, one SPMD Bass program, per-core data via inputs):
  - Routing (sigmoid gating + top-2 + weight normalization) runs on host with
    the exact jax CPU ops of the reference: the random centroids saturate the
    sigmoid, producing thousands of exact ties broken by expert index, so any
    approximate device sigmoid (ACT LUT) flips selections. Routing is 0.13% of
    total FLOPs; all 206 GFLOP of FFN compute runs on device in fp32r (full
    78.6 TF/s PE rate on fp32 data, ~3e-4 relative error).
  - Expert-parallel FFN with static load balancing: each core gets three
    "pieces" of capacity [768, 384, 128] token-slots (1280 slots/core). Expert
    token lists are carved into these pieces; hot experts are split across
    cores with their tokens dealt capacity-proportionally across the pieces
    (_stride_split_experts) so every piece's contributions spread evenly over
    all owner cores, minimizing the AllToAll bucket capacity (SCAP = the mean
    load). Piece-to-core pairing is further optimized (_optimize_pairing) to
    flatten per-(core, owner) maxima and co-locate same-expert pieces.
  - The host pre-gathers and pre-transposes each core's tokens (dispatch-side
    sharding), so the device does pure dense SwiGLU: up/gate with stationary
    weights, transpose-free down-projection (h as the stationary operand,
    emitting token-major output directly), rows scaled by combine weight on
    PSUM eviction, then one 128-row indirect scatter per slot-tile into the
    owner-bucketed send buffer.
  - One AllToAll moves contributions to token-owner cores (bandwidth-bound,
    NRT barriers it against compute). Owners indirect-gather their two
    contributions per token, add the sequence-parallel shared-expert output
    (emitted after the routed phase so its PE work can overlap the collective
    and routed tail), and write the final [512, 2048] slice; the host
    concatenates the 8 slices.
"""

import os
import numpy as np

B, S, H = 4, 1024, 2048
E = 16
TOPK = 2
I = 1024
ISH = 2048
RATIO = 0.1
EPS = 1e-9
T = B * S
NC = 8
P = 128
TOWN = T // NC  # 512 tokens owned per core
PIECE_SIZES = (768, 384, 128)
CAP = sum(PIECE_SIZES)  # 1280 slots per core
N_TILES = CAP // P  # 10
KC_H = H // P    # 16
M_I = I // P     # 8
M_ISH = ISH // P  # 16
DUMMY_TOK = T  # extra zero row in x_pad
BIG = 10 ** 9

_COMPILED = {}
SKIP_PHASES = frozenset()  # debug: subsets of {'shared','routed','a2a','combine'}


def _enable_jax_cache():
    import jax
    try:
        cache_dir = os.environ.get("KERNEL_JAX_CACHE", "/tmp/jax_moe_cache")
        jax.config.update("jax_compilation_cache_dir", cache_dir)
        jax.config.update("jax_persistent_cache_min_compile_time_secs", 0.0)
    except Exception:
        pass


def _host_routing(x, centroids, gate_bias):
    """Bit-identical routing to the reference (jax CPU ops)."""
    import jax
    import jax.numpy as jnp
    cpu = jax.devices("cpu")[0]
    with jax.default_device(cpu):
        xj = jax.device_put(np.asarray(x), cpu)
        cj = jax.device_put(np.asarray(centroids), cpu)
        gj = jax.device_put(np.asarray(gate_bias), cpu)
        aff = jax.nn.sigmoid(jnp.einsum('bsh,eh->bse', xj, cj))
        biased = aff + gj
        _, top_idx = jax.lax.top_k(biased, TOPK)
        top_aff = jnp.take_along_axis(aff, top_idx, axis=-1)
        weights = top_aff / (top_aff.sum(-1, keepdims=True) + EPS)
    top_idx = np.asarray(top_idx).reshape(T, TOPK).astype(np.int64)
    weights = np.asarray(weights).reshape(T, TOPK).astype(np.float32)
    return top_idx, weights


def _assign_pieces(counts):
    """Carve expert token lists into pieces of sizes 768/384/128 (8 of each),
    then pack one piece of each size per core, co-locating same-expert pieces
    to minimize weight traffic. Returns per-core piece lists
    [(expert, offset_in_expert_list, realcount, size), ...] ordered [A,B,C]."""
    avail = {768: 8, 384: 8, 128: 8}
    pieces = {768: [], 384: [], 128: []}
    order = np.argsort(-np.asarray(counts), kind="stable")
    for e in order:
        rem = int(counts[e])
        off = 0
        if rem == 0:
            continue
        while rem > 0:
            if rem > 384 and avail[768] > 0:
                sz = 768
            elif rem > 128 and avail[384] > 0:
                sz = 384
            elif rem <= 128 and avail[128] > 0:
                sz = 128
            elif avail[384] > 0:
                sz = 384
            elif avail[768] > 0:
                sz = 768
            else:
                raise RuntimeError("piece inventory exhausted; routing distribution unexpected")
            avail[sz] -= 1
            take = min(rem, sz)
            pieces[sz].append((int(e), off, take, sz))
            off += take
            rem -= take
    # dummy pieces for unused inventory
    for sz in (768, 384, 128):
        while avail[sz] > 0:
            pieces[sz].append((0, 0, 0, sz))
            avail[sz] -= 1
    # pack cores: one piece of each size; prefer same-expert grouping
    cores = []
    used_b = [False] * 8
    used_c = [False] * 8
    for a in pieces[768]:
        grp = [a]
        be = next((j for j, bp in enumerate(pieces[384])
                   if not used_b[j] and bp[2] > 0 and bp[0] == a[0]), None)
        if be is None:
            be = next(j for j, _ in enumerate(pieces[384]) if not used_b[j])
        used_b[be] = True
        grp.append(pieces[384][be])
        exps = {a[0], pieces[384][be][0]}
        ce = next((j for j, cp in enumerate(pieces[128])
                   if not used_c[j] and cp[2] > 0 and cp[0] in exps), None)
        if ce is None:
            ce = next(j for j, _ in enumerate(pieces[128]) if not used_c[j])
        used_c[ce] = True
        grp.append(pieces[128][ce])
        cores.append(grp)
    return cores



def _optimize_pairing(cores, lists):
    """Re-pair B/C pieces across cores to flatten the per-(core, owner)
    contribution-count maxima, which sets the AllToAll bucket capacity."""
    import itertools

    def owner_vec(piece):
        e, off, cnt, sz = piece
        v = np.zeros(NC, np.int64)
        for j in range(cnt):
            v[lists[e][off + j] // TOWN] += 1
        return v

    def max_bucket(groups):
        return max(int(sum((owner_vec(p) for p in g), np.zeros(NC, np.int64)).max())
                   for g in groups)

    A = [g[0] for g in cores]
    Bp = [g[1] for g in cores]
    Cp = [g[2] for g in cores]
    Av = [owner_vec(p) for p in A]
    Bv = [owner_vec(p) for p in Bp]
    Cv = [owner_vec(p) for p in Cp]
    orderA = sorted(range(NC), key=lambda i: -Av[i].max())
    availB = list(range(NC))
    assignB = {}
    for i in orderA:
        j = min(availB, key=lambda j: (Av[i] + Bv[j]).max())
        assignB[i] = j
        availB.remove(j)
    mid = {i: Av[i] + Bv[assignB[i]] for i in range(NC)}
    orderA2 = sorted(range(NC), key=lambda i: -mid[i].max())
    availC = list(range(NC))
    assignC = {}
    for i in orderA2:
        j = min(availC, key=lambda j: (mid[i] + Cv[j]).max())
        assignC[i] = j
        availC.remove(j)
    best = [[A[i], Bp[assignB[i]], Cp[assignC[i]]] for i in range(NC)]
    for _ in range(50):
        improved = False
        for (i, j) in itertools.combinations(range(NC), 2):
            for slot in (1, 2):
                g = [list(x) for x in best]
                g[i][slot], g[j][slot] = g[j][slot], g[i][slot]
                if max_bucket(g) < max_bucket(best):
                    best = g
                    improved = True
        if not improved:
            break
    return [tuple(g) for g in best] if max_bucket(best) < max_bucket(cores) else cores



def _stride_split_experts(cores, lists, wvals):
    """For experts split across multiple pieces, deal their tokens to the
    pieces capacity-proportionally instead of contiguously. Token order equals
    owner order, so contiguous carving concentrates each piece's contributions
    on few owners and inflates the AllToAll bucket capacity; dealing spreads
    every piece across all owners."""
    by_expert = {}
    for c in range(len(cores)):
        for pi, (e, off, cnt, sz) in enumerate(cores[c]):
            if cnt > 0:
                by_expert.setdefault(e, []).append((c, pi, off, cnt, sz))
    new_cores = [list(g) for g in cores]
    for e, ps in by_expert.items():
        if len(ps) < 2:
            continue
        ps.sort(key=lambda t: t[2])  # original carve order by offset
        n = sum(cnt for (_, _, _, cnt, _) in ps)
        caps = [sz for (_, _, _, _, sz) in ps]
        fills = [0] * len(ps)
        buckets = [[] for _ in ps]
        for j in range(n):
            k = min((i for i in range(len(ps)) if fills[i] < caps[i]),
                    key=lambda i: fills[i] / caps[i])
            buckets[k].append(j)
            fills[k] += 1
        perm = [j for b in buckets for j in b]
        lists[e] = [lists[e][j] for j in perm]
        wvals[e] = [wvals[e][j] for j in perm]
        off = 0
        for i, (c, pi, _, _, sz) in enumerate(ps):
            new_cores[c][pi] = (e, off, fills[i], sz)
            off += fills[i]
    return [tuple(g) for g in new_cores], lists, wvals


def _build_program():
    """Build the SPMD Bass program (same for all cores)."""
    import concourse.bass as bass
    import concourse.mybir as mybir
    import concourse.tile as tile
    from concourse import bacc
    from concourse.masks import make_identity

    dt = mybir.dt
    AF = mybir.ActivationFunctionType
    ALU = mybir.AluOpType

    SCAP = _build_program.SCAP
    SEND_ROWS = NC * SCAP

    nc = bacc.Bacc("TRN2", target_bir_lowering=False, num_devices=NC)

    f32, f32r, i32 = dt.float32, dt.float32r, dt.int32

    xg_in = nc.dram_tensor("xg_in", [KC_H, P, CAP], f32, kind="ExternalInput")
    wslot = nc.dram_tensor("wslot", [N_TILES, P], f32, kind="ExternalInput")
    send_pos = nc.dram_tensor("send_pos", [N_TILES, P], i32, kind="ExternalInput")
    recv_idx = nc.dram_tensor("recv_idx", [2, TOWN // P, P], i32, kind="ExternalInput")
    wg_in = nc.dram_tensor("wg_in", [3, M_I, P, KC_H, P], f32, kind="ExternalInput")
    wu_in = nc.dram_tensor("wu_in", [3, M_I, P, KC_H, P], f32, kind="ExternalInput")
    wd_in = nc.dram_tensor("wd_in", [3, M_I, P, H], f32, kind="ExternalInput")
    wgs_in = nc.dram_tensor("wgs_in", [M_ISH, P, KC_H, P], f32, kind="ExternalInput")
    wus_in = nc.dram_tensor("wus_in", [M_ISH, P, KC_H, P], f32, kind="ExternalInput")
    wds_in = nc.dram_tensor("wds_in", [M_ISH, P, H], f32, kind="ExternalInput")
    xT_own = nc.dram_tensor("xT_own", [KC_H, P, TOWN], f32, kind="ExternalInput")

    out_own = nc.dram_tensor("out_own", [TOWN, H], f32, kind="ExternalOutput")

    send_buf = nc.dram_tensor("send_buf", [SEND_ROWS, H], f32)
    recv_buf = nc.dram_tensor("recv_buf", [SEND_ROWS, H], f32)

    # piece -> (local tile offset, number of slot tiles, matmul blocks)
    piece_tiles = [sz // P for sz in PIECE_SIZES]
    piece_tile_off = [0, 6, 9]
    piece_blocks = {0: [(0, 512), (512, 256)], 1: [(0, 384)], 2: [(0, 128)]}

    with tile.TileContext(nc) as tc:
        with (
            tc.tile_pool(name="const", bufs=1) as constp,
            tc.tile_pool(name="big", bufs=1) as bigp,
            tc.tile_pool(name="io", bufs=2) as iop,
        ):
            shared_tok = bigp.tile([P, TOWN // P, H], f32, name="shared_tok")
            n_hb = H // 512

            # ---------------- routed experts: 3 pieces ----------------
            if "routed" not in SKIP_PHASES:
              with (
                  tc.tile_pool(name="rtbig", bufs=1) as rtbig,
                  tc.tile_pool(name="rtw", bufs=2) as rtw,
                  tc.tile_pool(name="rtwork", bufs=2) as work,
              ):
                  for p_i in range(3):
                      n_t = piece_tiles[p_i]
                      t_off = piece_tile_off[p_i]

                      up_ps = tc.tile_pool(name=f"upps{p_i}", bufs=1, space="PSUM")
                      psp = up_ps.__enter__()

                      # load pre-gathered, pre-transposed tokens for this piece
                      xgT = rtbig.tile([P, KC_H, 768], f32r, name="xgT", tag="xgT")
                      sz_p = PIECE_SIZES[p_i]
                      for kc in range(KC_H):
                          nc.sync.dma_start(
                              xgT[:, kc, :sz_p],
                              xg_in[kc, :, t_off * P:t_off * P + sz_p].bitcast(f32r))
                      wts = []
                      sidx = []
                      for st in range(n_t):
                          w_t = constp.tile([P, 1], f32, name=f"w_t{p_i}_{st}", tag=f"w_t{t_off + st}")
                          nc.sync.dma_start(w_t[:], wslot[t_off + st][:, None])
                          wts.append(w_t)
                          si_t = constp.tile([P, 1], i32, name=f"si_t{p_i}_{st}", tag=f"si_t{t_off + st}")
                          nc.sync.dma_start(si_t[:], send_pos[t_off + st][:, None])
                          sidx.append(si_t)

                      # up/gate projections -> h [i, slots] f32r
                      h = rtbig.tile([P, M_I, 768], f32r, name="h", tag="h")
                      for m in range(M_I):
                          wg_t = rtw.tile([P, KC_H, P], f32r, name="wg_t", tag="wg_t")
                          wu_t = rtw.tile([P, KC_H, P], f32r, name="wu_t", tag="wu_t")
                          if "wdma" not in SKIP_PHASES:
                              nc.sync.dma_start(wg_t[:], wg_in[p_i, m].bitcast(f32r))
                              nc.sync.dma_start(wu_t[:], wu_in[p_i, m].bitcast(f32r))
                          for (b0, bn) in piece_blocks[p_i]:
                              if "mm" in SKIP_PHASES:
                                  continue
                              psg2 = psp.tile([P, 512], f32, name="psg2", tag="psg", bufs=2)
                              psu2 = psp.tile([P, 512], f32, name="psu2", tag="psu", bufs=2)
                              for kc in range(KC_H):
                                  nc.tensor.matmul(psg2[:, :bn], wg_t[:, kc, :],
                                                   xgT[:, kc, b0:b0 + bn],
                                                   start=(kc == 0), stop=(kc == KC_H - 1))
                              for kc in range(KC_H):
                                  nc.tensor.matmul(psu2[:, :bn], wu_t[:, kc, :],
                                                   xgT[:, kc, b0:b0 + bn],
                                                   start=(kc == 0), stop=(kc == KC_H - 1))
                              sg2 = work.tile([P, 512], f32r, name="sg2", tag="sg2")
                              nc.scalar.activation(sg2[:, :bn], psg2[:, :bn], AF.Silu)
                              nc.vector.tensor_mul(h[:, m, b0:b0 + bn], sg2[:, :bn], psu2[:, :bn])

                      # down projection, token-major out; scale; scatter to send_buf
                      up_ps.__exit__(None, None, None)
                      dn_ps = tc.tile_pool(name=f"dnps{p_i}", bufs=1, space="PSUM")
                      dpsp = dn_ps.__enter__()
                      y_tok = [rtbig.tile([P, H], f32, name=f"y_tok{st}", tag=f"y_tok{st}")
                               for st in range(n_t)]
                      for hb in range(n_hb):
                          ps_d = [dpsp.tile([P, 512], f32, name=f"ps_d{st}", tag=f"ps_d{st}")
                                  for st in range(n_t)]
                          for ic in range(M_I):
                              wd_t = rtw.tile([P, 512], f32r, name="wd_t", tag="wd_t", bufs=4)
                              if "wdma" not in SKIP_PHASES:
                                  nc.sync.dma_start(wd_t[:], wd_in[p_i, ic][:, hb * 512:(hb + 1) * 512].bitcast(f32r))
                              for st in range(n_t):
                                  nc.tensor.matmul(ps_d[st][:], h[:, ic, st * P:(st + 1) * P],
                                                   wd_t[:], start=(ic == 0), stop=(ic == M_I - 1))
                          for st in range(n_t):
                              nc.vector.tensor_scalar_mul(
                                  y_tok[st][:, hb * 512:(hb + 1) * 512],
                                  ps_d[st][:], wts[st][:, :1])
                      for st in range(n_t):
                          if "scatter" in SKIP_PHASES:
                              continue
                          nc.gpsimd.indirect_dma_start(
                              out=send_buf[:, :], in_=y_tok[st][:],
                              out_offset=bass.IndirectOffsetOnAxis(ap=sidx[st][:, :1], axis=0),
                              in_offset=None,
                              bounds_check=SEND_ROWS - 1,
                              oob_is_err=False)
                      dn_ps.__exit__(None, None, None)

            # ---------------- all-to-all combine ----------------
            if "a2a" not in SKIP_PHASES:
              nc.gpsimd.collective_compute(
                "AllToAll",
                mybir.AluOpType.bypass,
                replica_groups=[list(range(NC))],
                ins=[send_buf[:, :].opt()],
                outs=[recv_buf[:, :].opt()],
              )

            # ---------------- shared expert (own 512 tokens) ----------------
            if "shared" in SKIP_PHASES:
                nc.vector.memset(shared_tok[:], 0.0)
            else:
              with (
                  tc.tile_pool(name="shbig", bufs=1) as shbig,
                  tc.tile_pool(name="shw", bufs=2) as shw,
                  tc.tile_pool(name="shps", bufs=1, space="PSUM") as psp,
              ):
                  xTo = shbig.tile([P, KC_H, TOWN], f32r, name="xTo")
                  nc.sync.dma_start(xTo[:], xT_own.rearrange("kc p t -> p kc t").bitcast(f32r))

                  hs = shbig.tile([P, M_ISH, TOWN], f32r, name="hs")
                  for m in range(M_ISH):
                      wgs_t = shw.tile([P, KC_H, P], f32r, name="wgs_t", tag="wgs_t", bufs=4)
                      wus_t = shw.tile([P, KC_H, P], f32r, name="wus_t", tag="wus_t", bufs=4)
                      nc.sync.dma_start(wgs_t[:], wgs_in[m].bitcast(f32r))
                      nc.sync.dma_start(wus_t[:], wus_in[m].bitcast(f32r))
                      psg = psp.tile([P, TOWN], f32, name="psg", tag="psg", bufs=2)
                      psu = psp.tile([P, TOWN], f32, name="psu", tag="psu", bufs=2)
                      for kc in range(KC_H):
                          nc.tensor.matmul(psg[:], wgs_t[:, kc, :], xTo[:, kc, :],
                                           start=(kc == 0), stop=(kc == KC_H - 1))
                      for kc in range(KC_H):
                          nc.tensor.matmul(psu[:], wus_t[:, kc, :], xTo[:, kc, :],
                                           start=(kc == 0), stop=(kc == KC_H - 1))
                      sg = shw.tile([P, TOWN], f32r, name="sg", tag="sg")
                      nc.scalar.activation(sg[:], psg[:], AF.Silu)
                      nc.vector.tensor_mul(hs[:, m, :], sg[:], psu[:])

                  # shared down-projection, output token-major directly
                  for hb in range(n_hb):
                      ps_sh = [psp.tile([P, 512], f32, name=f"ps_sh{tt}", tag=f"ps_sh{tt}")
                               for tt in range(TOWN // P)]
                      for ic in range(M_ISH):
                          wds_t = shw.tile([P, 512], f32r, name="wds_t", tag="wds_t", bufs=4)
                          nc.sync.dma_start(wds_t[:], wds_in[ic][:, hb * 512:(hb + 1) * 512].bitcast(f32r))
                          for tt in range(TOWN // P):
                              nc.tensor.matmul(ps_sh[tt][:], hs[:, ic, tt * P:(tt + 1) * P],
                                               wds_t[:], start=(ic == 0), stop=(ic == M_ISH - 1))
                      for tt in range(TOWN // P):
                          nc.scalar.activation(shared_tok[:, tt, hb * 512:(hb + 1) * 512],
                                               ps_sh[tt][:], AF.Copy, scale=RATIO)


            with tc.tile_pool(name="cmb", bufs=2) as cmb:
                for tt in range(TOWN // P):
                    i1 = iop.tile([P, 1], i32, name="i1", tag="i1")
                    i2 = iop.tile([P, 1], i32, name="i2", tag="i2")
                    nc.sync.dma_start(i1[:], recv_idx[0, tt][:, None])
                    nc.sync.dma_start(i2[:], recv_idx[1, tt][:, None])
                    g1 = cmb.tile([P, H], f32, name="g1", tag="g1")
                    g2 = cmb.tile([P, H], f32, name="g2", tag="g2")
                    nc.gpsimd.indirect_dma_start(
                        out=g1[:], out_offset=None, in_=recv_buf[:, :],
                        in_offset=bass.IndirectOffsetOnAxis(ap=i1[:, :1], axis=0))
                    nc.gpsimd.indirect_dma_start(
                        out=g2[:], out_offset=None, in_=recv_buf[:, :],
                        in_offset=bass.IndirectOffsetOnAxis(ap=i2[:, :1], axis=0))
                    nc.vector.tensor_add(g1[:], g1[:], g2[:])
                    nc.vector.tensor_add(g1[:], g1[:], shared_tok[:, tt, :])
                    nc.sync.dma_start(out_own[tt * P:(tt + 1) * P, :], g1[:])

    nc.finalize()
    return nc


def prepare_in_maps(x, centroids, gate_bias, wg_s, wu_s, wd_s, wg, wu, wd):
    x = np.ascontiguousarray(np.asarray(x, dtype=np.float32))
    wg = np.asarray(wg, dtype=np.float32)
    wu = np.asarray(wu, dtype=np.float32)
    wd = np.asarray(wd, dtype=np.float32)

    top_idx, weights = _host_routing(x, centroids, gate_bias)

    # expert token lists in token order
    lists = [[] for _ in range(E)]
    wvals = [[] for _ in range(E)]
    for t in range(T):
        for k in range(TOPK):
            e = int(top_idx[t, k])
            lists[e].append(t)
            wvals[e].append(weights[t, k])
    counts = [len(l) for l in lists]
    cores = _assign_pieces(counts)
    cores, lists, wvals = _stride_split_experts(cores, lists, wvals)
    cores = _optimize_pairing(cores, lists)

    # per-core slot tables
    tok_ids = np.full((NC, N_TILES, P), DUMMY_TOK, dtype=np.int32)
    wslot = np.zeros((NC, N_TILES, P), dtype=np.float32)
    piece_expert = np.zeros((NC, 3), dtype=np.int64)
    for c in range(NC):
        loc = 0
        for pi, (e, off, cnt, sz) in enumerate(cores[c]):
            piece_expert[c, pi] = e
            pts = [(lists[e][off + j], wvals[e][off + j]) for j in range(cnt)]
            pts.sort(key=lambda tw: (tw[0] // TOWN, tw[0]))
            for j, (t, w) in enumerate(pts):
                tok_ids[c, (loc + j) // P, (loc + j) % P] = t
                wslot[c, (loc + j) // P, (loc + j) % P] = w
            loc += sz

    # send positions / recv indices
    cnt_co = np.zeros((NC, NC), dtype=np.int64)
    contrib = [[] for _ in range(T)]  # (core, pos) per contribution
    for c in range(NC):
        for loc in range(CAP):
            t = int(tok_ids[c, loc // P, loc % P])
            if t == DUMMY_TOK:
                continue
            o = t // TOWN
            pos = cnt_co[c, o]
            cnt_co[c, o] += 1
            contrib[t].append((c, int(pos)))
    SCAP = int(((cnt_co.max() + 15) // 16) * 16)
    # destination row = owner * SCAP + pos
    send_pos_arr = np.full((NC, N_TILES, P), BIG, dtype=np.int32)
    cnt_co2 = np.zeros((NC, NC), dtype=np.int64)
    for c in range(NC):
        for loc in range(CAP):
            t = int(tok_ids[c, loc // P, loc % P])
            if t == DUMMY_TOK:
                continue
            o = t // TOWN
            pos = cnt_co2[c, o]
            cnt_co2[c, o] += 1
            send_pos_arr[c, loc // P, loc % P] = o * SCAP + pos

    recv_idx = np.zeros((NC, 2, TOWN // P, P), dtype=np.int32)
    for t in range(T):
        o = t // TOWN
        tl = t % TOWN
        assert len(contrib[t]) == 2, (t, contrib[t])
        for k, (c, pos) in enumerate(contrib[t]):
            recv_idx[o, k, tl // P, tl % P] = c * SCAP + pos

    # weight tensors, matmul-ready tiling
    def tile_up(w2d, mm):  # [H, mm*128] -> [mm, 128, KC_H, 128]
        return np.ascontiguousarray(
            w2d.reshape(KC_H, P, mm, P).transpose(2, 1, 0, 3))

    def tile_dn(w2d, mm):  # [mm*128, H] -> [mm, 128, H]
        return np.ascontiguousarray(w2d.reshape(mm, P, H))

    wg_t = np.zeros((NC, 3, M_I, P, KC_H, P), dtype=np.float32)
    wu_t = np.zeros((NC, 3, M_I, P, KC_H, P), dtype=np.float32)
    wd_t = np.zeros((NC, 3, M_I, P, H), dtype=np.float32)
    done = {}
    for c in range(NC):
        for pi, (e, off, cnt, sz) in enumerate(cores[c]):
            if cnt == 0:
                continue
            if e not in done:
                done[e] = (tile_up(wg[e], M_I), tile_up(wu[e], M_I), tile_dn(wd[e], M_I))
            wg_t[c, pi], wu_t[c, pi], wd_t[c, pi] = done[e]

    wgs_t = tile_up(np.asarray(wg_s, np.float32), M_ISH)
    wus_t = tile_up(np.asarray(wu_s, np.float32), M_ISH)
    wds_t = tile_dn(np.asarray(wd_s, np.float32), M_ISH)

    x_flat = x.reshape(T, H)
    x_pad = np.vstack([x_flat, np.zeros((1, H), np.float32)])

    in_maps = []
    for c in range(NC):
        xo = np.ascontiguousarray(
            x_flat[c * TOWN:(c + 1) * TOWN].T.reshape(KC_H, P, TOWN))
        xg_c = np.ascontiguousarray(
            x_pad[tok_ids[c].reshape(-1)].T.reshape(KC_H, P, CAP))
        in_maps.append({
            "xg_in": xg_c,
            "wslot": wslot[c],
            "send_pos": send_pos_arr[c],
            "recv_idx": recv_idx[c],
            "wg_in": wg_t[c],
            "wu_in": wu_t[c],
            "wd_in": wd_t[c],
            "wgs_in": wgs_t,
            "wus_in": wus_t,
            "wds_in": wds_t,
            "xT_own": xo,
        })

    return in_maps, SCAP


def get_program(scap):
    key = ("moe", scap)
    if key not in _COMPILED:
        _build_program.SCAP = scap
        _COMPILED[key] = _build_program()
    return _COMPILED[key]


_RUNNER = {}


def _build_runner(nc, n_cores=NC):
    """Build a reusable PJRT executable for the finalized Bass program.
    Mirrors concourse.bass2jax.run_bass_via_pjrt but without output donation,
    so the jitted callable can be invoked repeatedly and its HLO is stable
    across processes (persistent-cache friendly)."""
    import jax
    import concourse.mybir as mybir
    from concourse import bass2jax as b2j
    from jax.experimental.shard_map import shard_map
    from jax.sharding import Mesh, PartitionSpec, NamedSharding

    b2j.install_neuronx_cc_hook()
    partition_name = nc.partition_id_tensor.name if nc.partition_id_tensor else None
    in_names, out_names, out_avals, zero_outs = [], [], [], []
    for alloc in nc.m.functions[0].allocations:
        if not isinstance(alloc, mybir.MemoryLocationSet):
            continue
        name = alloc.memorylocations[0].name
        if alloc.kind == "ExternalInput":
            if name != partition_name:
                in_names.append(name)
        elif alloc.kind == "ExternalOutput":
            shape = tuple(alloc.tensor_shape)
            dtype = mybir.dt.np(alloc.dtype)
            out_avals.append(jax.core.ShapedArray(shape, dtype))
            out_names.append(name)
            zero_outs.append(np.zeros(shape, dtype))
    n_params = len(in_names)
    all_in_names = in_names + out_names
    if partition_name is not None:
        all_in_names = all_in_names + [partition_name]

    def _body(*args):
        operands = list(args)
        if partition_name is not None:
            operands.append(b2j.partition_id_tensor())
        outs = b2j._bass_exec_p.bind(
            *operands,
            out_avals=tuple(out_avals),
            in_names=tuple(all_in_names),
            out_names=tuple(out_names),
            lowering_input_output_aliases=(),
            sim_require_finite=True,
            sim_require_nnan=True,
            nc=nc,
        )
        return tuple(outs)

    devices = jax.devices()[:n_cores]
    mesh = Mesh(np.asarray(devices), ("core",))
    spec = PartitionSpec("core")
    sharded = jax.jit(
        shard_map(_body, mesh=mesh, in_specs=(spec,) * (n_params + len(out_names)),
                  out_specs=(spec,) * len(out_names), check_rep=False),
        keep_unused=True,
    )
    sh = NamedSharding(mesh, spec)

    def run(in_maps):
        concat_in = [
            np.concatenate([np.asarray(in_maps[c][nm]) for c in range(n_cores)], axis=0)
            for nm in in_names
        ]
        concat_zeros = [np.zeros((n_cores * z.shape[0], *z.shape[1:]), z.dtype)
                        for z in zero_outs]
        dev_in = [jax.device_put(a, sh) for a in concat_in]
        dev_zero = [jax.device_put(a, sh) for a in concat_zeros]
        out = sharded(*dev_in, *dev_zero)
        jax.block_until_ready(out)
        return ({nm: np.asarray(out[i]) for i, nm in enumerate(out_names)},
                (sharded, dev_in, dev_zero))

    return run


def kernel(x, centroids, gate_bias, wg_s, wu_s, wd_s, wg, wu, wd):
    _enable_jax_cache()
    in_maps, scap = prepare_in_maps(x, centroids, gate_bias, wg_s, wu_s, wd_s, wg, wu, wd)
    nc = get_program(scap)
    key = ("run", scap)
    if key not in _RUNNER:
        _RUNNER[key] = _build_runner(nc)
    outs, _ = _RUNNER[key](in_maps)
    out = outs["out_own"].reshape(NC, TOWN, H)
    return np.ascontiguousarray(out.reshape(B, S, H))

